# revision 5
# baseline (speedup 1.0000x reference)
"""Trainium2 Bass kernel for GNN message passing (APPR-style aggregation).

Computes: out = x + 0.15 * segment_sum(x[src], dst, num_segments=N)
for x [100000, 64] f32 and edge_index [2, 1600000] int64.

Strategy (8 NeuronCores, no collectives needed):
  - Host shards EDGES by destination-owner core (core c owns nodes
    [c*12500, (c+1)*12500)); within a core, edges are bucketed by
    128-node destination block and by source quadrant (x split into 4
    row-quadrants so dma_gather's int16 indices can address it). This
    makes the aggregation node-sharded from the start, so each core
    independently produces its slice of the output.
  - On device, per 128-node block: dma_gather of x[src] rows into SBUF
    (one gather per source quadrant), build one-hot selection matrices
    S[e, j] = (dstcol_e == j) with a DVE is_equal against an iota tile,
    and accumulate S^T @ gathered into a PSUM tile over all edge tiles
    of the block. Epilogue adds the x slice (host pre-scales x by 0.15
    so the matmul result needs no extra scaling) and DMAs the block out.
  - All 8 cores run the same static graph: per-(block, quadrant) tile
    counts are maxed across cores and each core's edge lists are padded
    (pad slots gather row 0 of the quadrant and carry dstcol=255 which
    matches no one-hot column, so they contribute zero).
"""

import os
import sys
import types

import numpy as np

for _p in ("/opt/trn_rl_repo", "/root/.axon_site/_ro/trn_rl_repo"):
    if os.path.isdir(_p) and _p not in sys.path:
        sys.path.append(_p)

import concourse.bass as bass
import concourse.mybir as mybir
import concourse.tile as tile
from concourse import bacc
from concourse.bass_utils import run_bass_kernel_spmd
from concourse.vector_clock import ScopedClock

WEIGHT = 0.15
N_NODES = 100000
D_FEAT = 64
N_CORES = 8
P = 128
NQUAD = 4
NPC = N_NODES // N_CORES  # nodes per core
NBLK = (NPC + P - 1) // P  # 128-node dst blocks per core
NQROWS = N_NODES // NQUAD  # rows per source quadrant (must fit int16)

LAST_EXEC_TIME_NS = None

MAX_WAITS = 2  # this walrus build rejects instructions with more sync commands


def _patch_tile_drain():
    """This walrus build rejects >MAX_WAITS sync commands (waits+updates)
    on one instruction. Two patches: (a) the tail drain re-emits its waits
    as individual wait_ge instructions; (b) any scheduled instruction with
    too many waits gets the excess hoisted onto same-engine InstNoOps
    placed immediately before it."""
    if getattr(tile.TileContext, "_drain_patched", False):
        return

    def _drain_and_barrier(self, tick_clock, wait_clock):
        drain_inst = self.nc.sync.drain()
        wait_clock.add_sem_waits(
            drain_inst.ins, ScopedClock({None: tick_clock.global_clock})
        )
        si = drain_inst.ins.sync_info
        waits = list(si.on_wait) if si is not None else []
        if len(waits) > MAX_WAITS:
            drain_inst.ins.sync_info = mybir.SyncInfo(on_wait=[], on_update=[])
            handles = {h.name: h for h in wait_clock.sems.allocated().values()}
            for w in waits:
                self.nc.sync.wait_ge(handles[w.ant_name], w.wait_value)
            self.nc.sync.drain()
        self.nc.all_engine_barrier()
        popped = self.nc._tile_sem_poison_stack.pop()
        assert popped is self._sem_poison
        self.nc.clear_and_free_semaphores(list(self.sems.allocated().values()))
        self.nc.all_engine_barrier()

    orig_lower = tile.TileContext._lower_ordered_insts

    def _lower_ordered_insts(self, ordered):
        for bb_name, insts in ordered.items():
            new_list = []
            for inst in insts:
                si = getattr(inst, "sync_info", None)
                n_w = len(si.on_wait) if si is not None and si.on_wait else 0
                n_u = len(si.on_update) if si is not None and si.on_update else 0
                budget = max(0, MAX_WAITS - n_u)
                if (
                    n_w > budget
                    and type(inst).__name__.startswith("Inst")
                    and inst.engine is not None
                ):
                    waits = list(si.on_wait)
                    keep = waits[len(waits) - budget :] if budget else []
                    excess = waits[: len(waits) - budget]
                    for w in excess:
                        nop = mybir.InstNoOp(
                            name=self.nc.get_next_instruction_name(),
                            sync_info=mybir.SyncInfo(on_wait=[w], on_update=[]),
                            engine=inst.engine,
                            bass_nofuse=True,
                        )
                        new_list.append(nop)
                    inst.sync_info = mybir.SyncInfo(
                        on_wait=keep, on_update=list(si.on_update)
                    )
                new_list.append(inst)
            insts[:] = new_list
        return orig_lower(self, ordered)

    tile.TileContext._drain_and_barrier = _drain_and_barrier
    tile.TileContext._lower_ordered_insts = _lower_ordered_insts
    tile.TileContext._drain_patched = True


def _install_ntff_hook():
    """Register the NTFF profiling hook that this container's boot skips
    (antenv.axon_hooks missing). Only needed when tracing is requested."""
    if "antenv.axon_hooks" in sys.modules:
        return
    try:
        from trn_agent_boot.trn_boot import _ntff_profile_via_ctypes

        hook = _ntff_profile_via_ctypes("/opt/axon/libaxon_pjrt.so")
        if hook is None:
            return
        mod = types.ModuleType("antenv.axon_hooks")
        mod._hook = hook
        mod.get_axon_ntff_profile_hook = lambda: mod._hook
        mod.set_axon_ntff_profile_hook = lambda h: setattr(mod, "_hook", h)
        sys.modules["antenv.axon_hooks"] = mod
        import antenv

        antenv.axon_hooks = mod
    except Exception as e:  # profiling is optional
        print(f"ntff hook install failed: {e}", file=sys.stderr)


def _preprocess(x, edge_index):
    """Bucket edges per (core, dst-block, src-quadrant); build device
    input arrays in the slot order the device graph consumes."""
    src = np.asarray(edge_index[0]).astype(np.int64)
    dst = np.asarray(edge_index[1]).astype(np.int64)
    E = src.shape[0]

    core = dst // NPC
    local = dst - core * NPC
    blk = local >> 7
    col = local & 127
    quad = src // NQROWS
    loc = (src - quad * NQROWS).astype(np.int64)

    gkey = (core * NBLK + blk) * NQUAD + quad
    order = np.argsort(gkey, kind="stable")
    gkey_s = gkey[order]
    loc_s = loc[order]
    col_s = col[order]

    counts = np.bincount(gkey, minlength=N_CORES * NBLK * NQUAD).reshape(
        N_CORES, NBLK, NQUAD
    )
    # tiles per (block, quadrant), shared across cores
    tq = (counts.max(axis=0) + P - 1) // P  # [NBLK, NQUAD]
    for b in range(NBLK):
        if tq[b].sum() == 0:
            tq[b, 0] = 1
    tiles = tq.sum(axis=1)  # [NBLK] total tiles per block
    t_total = int(tiles.sum())

    # global tile offset of each (b, q) group
    toff = np.zeros((NBLK, NQUAD), dtype=np.int64)
    flat = tq.ravel()
    toff.ravel()[1:] = np.cumsum(flat)[:-1]

    group_starts = np.zeros(N_CORES * NBLK * NQUAD + 1, dtype=np.int64)
    np.cumsum(counts.ravel(), out=group_starts[1:])
    j = np.arange(E) - group_starts[gkey_s]  # slot within group
    bq_s = gkey_s % (NBLK * NQUAD)  # (b, q) linear
    core_s = gkey_s // (NBLK * NQUAD)
    base_tile = toff.ravel()[bq_s]

    # col array: [core, 128, t_total]; slot i -> lane i%128, tile base+i//128
    col_arr = np.full((N_CORES, P, t_total), 255.0, dtype=np.float32)
    col_arr[core_s, j & 127, base_tile + (j >> 7)] = col_s

    # idx array (int16, wrapped in 16 partitions, replicated x8):
    # slot i of group -> [i%16, o16 + i//16] with o16 = base_tile*8
    t16_total = t_total * 8
    idx16 = np.zeros((N_CORES, 16, t16_total), dtype=np.int16)
    idx16[core_s, j & 15, base_tile * 8 + (j >> 4)] = loc_s
    idx_arr = np.tile(idx16, (1, 8, 1))

    return idx_arr, col_arr, tq.astype(int), t_total


def _build_graph(tq, t_total):
    nc = bacc.Bacc()
    f32 = mybir.dt.float32
    xq_p = [
        nc.declare_dram_parameter(f"xq{q}", [NQROWS, D_FEAT], f32, isOutput=False)
        for q in range(NQUAD)
    ]
    xsl_p = nc.declare_dram_parameter("xsl", [NPC, D_FEAT], f32, isOutput=False)
    srci_p = nc.declare_dram_parameter(
        "srci", [P, t_total * 8], mybir.dt.int16, isOutput=False
    )
    dcol_p = nc.declare_dram_parameter("dcol", [P, t_total], f32, isOutput=False)
    iota_p = nc.declare_dram_parameter("iota", [P, P], f32, isOutput=False)
    out_p = nc.declare_dram_parameter("out", [NPC, D_FEAT], f32, isOutput=True)

    nblk = tq.shape[0]
    tiles = tq.sum(axis=1)
    t_max = int(tiles.max())

    with tile.TileContext(nc) as tc:
        with (
            tc.tile_pool(name="const", bufs=1) as const_tp,
            tc.tile_pool(name="meta", bufs=1) as meta_tp,
            tc.tile_pool(name="gather", bufs=3) as gather_tp,
            tc.tile_pool(name="sel", bufs=3) as sel_tp,
            tc.tile_pool(name="xin", bufs=3) as xin_tp,
            tc.tile_pool(name="osb", bufs=3) as osb_tp,
            tc.tile_pool(name="psum", bufs=4, space="PSUM") as psum_tp,
        ):
            iota_sb = const_tp.tile([P, P], f32)
            nc.sync.dma_start(out=iota_sb[:], in_=iota_p[:])
            idx_sb = meta_tp.tile([P, t_total * 8], mybir.dt.int16)
            nc.sync.dma_start(out=idx_sb[:], in_=srci_p[:])
            col_sb = meta_tp.tile([P, t_total], f32)
            nc.sync.dma_start(out=col_sb[:], in_=dcol_p[:])

            t0 = 0
            for b in range(nblk):
                tb = int(tiles[b])
                rows = min(P, NPC - b * P)

                g = gather_tp.tile([P, t_max * D_FEAT], f32, tag="g")
                off = 0
                for q in range(NQUAD):
                    tqn = int(tq[b, q])
                    if tqn == 0:
                        continue
                    o16 = (t0 + off) * 8
                    nc.gpsimd.dma_gather(
                        out_ap=g[
                            :, off * D_FEAT : (off + tqn) * D_FEAT
                        ].rearrange("p (c d) -> p c d", d=D_FEAT),
                        in_ap=xq_p[q][:, :],
                        idxs_ap=idx_sb[:, o16 : o16 + tqn * 8],
                        num_idxs=tqn * P,
                        num_idxs_reg=tqn * P,
                        elem_size=D_FEAT,
                    )
                    off += tqn

                sel = sel_tp.tile([P, t_max * P], f32, tag="s")
                nc.vector.tensor_tensor(
                    out=sel[:, : tb * P].rearrange("p (t n) -> p t n", n=P),
                    in0=col_sb[:, t0 : t0 + tb].unsqueeze(2).to_broadcast(
                        [P, tb, P]
                    ),
                    in1=iota_sb[:].unsqueeze(1).to_broadcast([P, tb, P]),
                    op=mybir.AluOpType.is_equal,
                )

                ps = psum_tp.tile([P, D_FEAT], f32, space="PSUM", tag="ps")
                for t in range(tb):
                    nc.tensor.matmul(
                        out=ps[:],
                        lhsT=sel[:, t * P : (t + 1) * P],
                        rhs=g[:, t * D_FEAT : (t + 1) * D_FEAT],
                        start=(t == 0),
                        stop=(t == tb - 1),
                    )

                xt = xin_tp.tile([P, D_FEAT], f32, tag="x")
                nc.sync.dma_start(out=xt[:rows], in_=xsl_p[b * P : b * P + rows, :])
                ot = osb_tp.tile([P, D_FEAT], f32, tag="o")
                nc.vector.tensor_add(out=ot[:rows], in0=xt[:rows], in1=ps[:rows])
                nc.sync.dma_start(out=out_p[b * P : b * P + rows, :], in_=ot[:rows])

                t0 += tb
    nc.compile()
    return nc


def kernel(x, edge_index):
    global LAST_EXEC_TIME_NS
    _patch_tile_drain()

    x = np.ascontiguousarray(np.asarray(x, dtype=np.float32))
    idx_arr, col_arr, tq, t_total = _preprocess(x, edge_index)

    x_scaled = (x * np.float32(WEIGHT)).astype(np.float32)
    iota = np.broadcast_to(np.arange(P, dtype=np.float32), (P, P)).copy()

    nc = _build_graph(tq, t_total)

    in_maps = []
    for c in range(N_CORES):
        m = {
            "xsl": np.ascontiguousarray(x[c * NPC : (c + 1) * NPC]),
            "srci": np.ascontiguousarray(idx_arr[c]),
            "dcol": np.ascontiguousarray(col_arr[c]),
            "iota": iota,
        }
        for q in range(NQUAD):
            m[f"xq{q}"] = np.ascontiguousarray(
                x_scaled[q * NQROWS : (q + 1) * NQROWS]
            )
        in_maps.append(m)

    trace = bool(os.environ.get("BASS_KERNEL_TRACE"))
    if trace:
        _install_ntff_hook()
    res = run_bass_kernel_spmd(
        nc, in_maps, core_ids=list(range(N_CORES)), trace=trace
    )
    LAST_EXEC_TIME_NS = res.exec_time_ns

    out = np.concatenate([res.results[c]["out"] for c in range(N_CORES)], axis=0)
    return out.astype(np.float32)


# revision 7
# speedup vs baseline: 3.3980x; 3.3980x over previous
"""Trainium2 Bass kernel for GNN message passing (APPR-style aggregation).

Computes: out = x + 0.15 * segment_sum(x[src], dst, num_segments=N)
for x [100000, 64] f32 and edge_index [2, 1600000] int64.

Strategy (8 NeuronCores, no collectives needed):
  - Host shards EDGES by destination-owner core (core c owns nodes
    [c*12500, (c+1)*12500)); within a core, edges are bucketed by
    128-node destination block and by source quadrant (x split into 4
    row-quadrants so dma_gather's int16 indices can address it). This
    makes the aggregation node-sharded from the start, so each core
    independently produces its slice of the output.
  - On device, per 128-node block: dma_gather of 0.15*x[src] rows (bf16,
    rows padded to 256 B) into SBUF, one gather per source quadrant,
    round-robined over all 4 SWDGE queues; build one-hot selection
    matrices S[e, j] = (dstcol_e == j) with a DVE is_equal against an
    iota tile, and accumulate S^T @ gathered into an f32 PSUM tile over
    all edge tiles of the block. Epilogue adds the f32 x slice and DMAs
    the block out.
  - All 8 cores run the same static graph: per-(block, quadrant) tile
    counts are maxed across cores; each core pads its edge list with
    valid index-0 entries up to the shared valid count and with -1
    entries (skipped by the gather) up to the tile boundary. Pad slots
    carry dstcol=255 which matches no one-hot column, so they contribute
    zero; slots the gather skips hold stale-but-finite SBUF data (the
    gather pool is memset once at startup).
"""

import os
import sys
import types

import numpy as np

for _p in ("/opt/trn_rl_repo", "/root/.axon_site/_ro/trn_rl_repo"):
    if os.path.isdir(_p) and _p not in sys.path:
        sys.path.append(_p)

import ml_dtypes
import concourse.bass as bass
import concourse.mybir as mybir
import concourse.tile as tile
from concourse import bacc
from concourse.bass_utils import run_bass_kernel_spmd
from concourse.vector_clock import ScopedClock

WEIGHT = 0.15
N_NODES = 100000
D_FEAT = 64
N_CORES = 8
P = 128
NQUAD = 4
ROWPAD = 128  # gathered bf16 row padded to 128 elems = 256 B
NPC = N_NODES // N_CORES  # nodes per core
NBLK = (NPC + P - 1) // P  # 128-node dst blocks per core
NQROWS = N_NODES // NQUAD  # rows per source quadrant (must fit int16)

LAST_EXEC_TIME_NS = None

MAX_WAITS = 2  # this walrus build rejects instructions with more sync commands


def _patch_tile_drain():
    """This walrus build rejects >MAX_WAITS sync commands (waits+updates)
    on one instruction. Two patches: (a) the tail drain re-emits its waits
    as individual wait_ge instructions; (b) any scheduled instruction with
    too many waits gets the excess hoisted onto same-engine InstNoOps
    placed immediately before it."""
    if getattr(tile.TileContext, "_drain_patched", False):
        return

    def _drain_and_barrier(self, tick_clock, wait_clock):
        drain_inst = self.nc.sync.drain()
        wait_clock.add_sem_waits(
            drain_inst.ins, ScopedClock({None: tick_clock.global_clock})
        )
        si = drain_inst.ins.sync_info
        waits = list(si.on_wait) if si is not None else []
        if len(waits) > MAX_WAITS:
            drain_inst.ins.sync_info = mybir.SyncInfo(on_wait=[], on_update=[])
            handles = {h.name: h for h in wait_clock.sems.allocated().values()}
            for w in waits:
                self.nc.sync.wait_ge(handles[w.ant_name], w.wait_value)
            self.nc.sync.drain()
        self.nc.all_engine_barrier()
        popped = self.nc._tile_sem_poison_stack.pop()
        assert popped is self._sem_poison
        self.nc.clear_and_free_semaphores(list(self.sems.allocated().values()))
        self.nc.all_engine_barrier()

    orig_lower = tile.TileContext._lower_ordered_insts

    def _lower_ordered_insts(self, ordered):
        for bb_name, insts in ordered.items():
            new_list = []
            for inst in insts:
                si = getattr(inst, "sync_info", None)
                n_w = len(si.on_wait) if si is not None and si.on_wait else 0
                n_u = len(si.on_update) if si is not None and si.on_update else 0
                budget = max(0, MAX_WAITS - n_u)
                if (
                    n_w > budget
                    and type(inst).__name__.startswith("Inst")
                    and inst.engine is not None
                ):
                    waits = list(si.on_wait)
                    keep = waits[len(waits) - budget :] if budget else []
                    excess = waits[: len(waits) - budget]
                    for w in excess:
                        nop = mybir.InstNoOp(
                            name=self.nc.get_next_instruction_name(),
                            sync_info=mybir.SyncInfo(on_wait=[w], on_update=[]),
                            engine=inst.engine,
                            bass_nofuse=True,
                        )
                        new_list.append(nop)
                    inst.sync_info = mybir.SyncInfo(
                        on_wait=keep, on_update=list(si.on_update)
                    )
                new_list.append(inst)
            insts[:] = new_list
        return orig_lower(self, ordered)

    tile.TileContext._drain_and_barrier = _drain_and_barrier
    tile.TileContext._lower_ordered_insts = _lower_ordered_insts
    tile.TileContext._drain_patched = True


def _install_ntff_hook():
    """Register the NTFF profiling hook that this container's boot skips
    (antenv.axon_hooks missing). Only needed when tracing is requested."""
    if "antenv.axon_hooks" in sys.modules:
        return
    try:
        from trn_agent_boot.trn_boot import _ntff_profile_via_ctypes

        hook = _ntff_profile_via_ctypes("/opt/axon/libaxon_pjrt.so")
        if hook is None:
            return
        mod = types.ModuleType("antenv.axon_hooks")
        mod._hook = hook
        mod.get_axon_ntff_profile_hook = lambda: mod._hook
        mod.set_axon_ntff_profile_hook = lambda h: setattr(mod, "_hook", h)
        sys.modules["antenv.axon_hooks"] = mod
        import antenv

        antenv.axon_hooks = mod
    except Exception as e:  # profiling is optional
        print(f"ntff hook install failed: {e}", file=sys.stderr)


def _preprocess(x, edge_index):
    """Bucket edges per (core, dst-block, src-quadrant); build device
    input arrays in the slot order the device graph consumes."""
    src = np.asarray(edge_index[0]).astype(np.int64)
    dst = np.asarray(edge_index[1]).astype(np.int64)
    E = src.shape[0]

    core = dst // NPC
    local = dst - core * NPC
    blk = local >> 7
    col = local & 127
    quad = src // NQROWS
    loc = (src - quad * NQROWS).astype(np.int64)

    gkey = (core * NBLK + blk) * NQUAD + quad
    order = np.argsort(gkey, kind="stable")
    gkey_s = gkey[order]
    loc_s = loc[order]
    col_s = col[order]

    counts = np.bincount(gkey, minlength=N_CORES * NBLK * NQUAD).reshape(
        N_CORES, NBLK, NQUAD
    )
    maxc = counts.max(axis=0)  # [NBLK, NQUAD] max edges over cores
    tq = (maxc + P - 1) // P  # tiles per (block, quadrant)
    for b in range(NBLK):
        if tq[b].sum() == 0:
            tq[b, 0] = 1
    tiles = tq.sum(axis=1)
    t_total = int(tiles.sum())
    vcnt = tq * P  # all slots valid (pad entries gather row 0)

    toff = np.zeros((NBLK, NQUAD), dtype=np.int64)
    toff.ravel()[1:] = np.cumsum(tq.ravel())[:-1]

    group_starts = np.zeros(N_CORES * NBLK * NQUAD + 1, dtype=np.int64)
    np.cumsum(counts.ravel(), out=group_starts[1:])
    j = np.arange(E) - group_starts[gkey_s]  # slot within group
    bq_s = gkey_s % (NBLK * NQUAD)
    core_s = gkey_s // (NBLK * NQUAD)
    base_tile = toff.ravel()[bq_s]

    # col array: [core, 128, t_total]; slot i -> lane i%128, tile base+i//128
    col_arr = np.full((N_CORES, P, t_total), 255.0, dtype=ml_dtypes.bfloat16)
    col_arr[core_s, j & 127, base_tile + (j >> 7)] = col_s.astype(
        ml_dtypes.bfloat16
    )

    # idx array (int16, wrapped in 16 partitions, replicated x8):
    # slot i of group -> [i%16, o16 + i//16] with o16 = base_tile*8.
    # Pad slots keep index 0 (a valid row; their dstcol=255 zeroes them).
    t16_total = t_total * 8
    idx16 = np.zeros((N_CORES, 16, t16_total), dtype=np.int16)
    idx16[core_s, j & 15, base_tile * 8 + (j >> 4)] = loc_s
    idx_arr = np.tile(idx16, (1, 8, 1))

    return idx_arr, col_arr, tq.astype(int), vcnt.astype(int), t_total


def _build_graph(tq, vcnt, t_total):
    nc = bacc.Bacc(num_swdge_queues=4)
    f32 = mybir.dt.float32
    bf16 = mybir.dt.bfloat16
    xq_p = [
        nc.declare_dram_parameter(f"xq{q}", [NQROWS, ROWPAD], bf16, isOutput=False)
        for q in range(NQUAD)
    ]
    xsl_p = nc.declare_dram_parameter("xsl", [NPC, D_FEAT], f32, isOutput=False)
    srci_p = nc.declare_dram_parameter(
        "srci", [P, t_total * 8], mybir.dt.int16, isOutput=False
    )
    dcol_p = nc.declare_dram_parameter("dcol", [P, t_total], bf16, isOutput=False)
    iota_p = nc.declare_dram_parameter("iota", [P, P], bf16, isOutput=False)
    out_p = nc.declare_dram_parameter("out", [NPC, D_FEAT], f32, isOutput=True)

    nblk = tq.shape[0]
    tiles = tq.sum(axis=1)
    t_max = int(tiles.max())

    with tile.TileContext(nc) as tc:
        with (
            tc.tile_pool(name="const", bufs=1) as const_tp,
            tc.tile_pool(name="meta", bufs=1) as meta_tp,
            tc.tile_pool(name="gather", bufs=4) as gather_tp,
            tc.tile_pool(name="sel", bufs=3) as sel_tp,
            tc.tile_pool(name="xin", bufs=3) as xin_tp,
            tc.tile_pool(name="osb", bufs=3) as osb_tp,
            tc.tile_pool(name="psum", bufs=4, space="PSUM") as psum_tp,
        ):
            iota_sb = const_tp.tile([P, P], bf16)
            nc.sync.dma_start(out=iota_sb[:], in_=iota_p[:])
            idx_sb = meta_tp.tile([P, t_total * 8], mybir.dt.int16)
            nc.sync.dma_start(out=idx_sb[:], in_=srci_p[:])
            col_sb = meta_tp.tile([P, t_total], bf16)
            nc.sync.dma_start(out=col_sb[:], in_=dcol_p[:])

            # memset the gather pool slots once: slots the -1-padded
            # gathers skip must hold finite data for the 0*x matmuls
            for _ in range(4):
                gz = gather_tp.tile([P, t_max * ROWPAD], bf16, tag="g")
                nc.vector.memset(gz[:], 0.0)

            gather_i = 0
            t0 = 0
            for b in range(nblk):
                tb = int(tiles[b])
                rows = min(P, NPC - b * P)

                g = gather_tp.tile([P, t_max * ROWPAD], bf16, tag="g")
                off = 0
                for q in range(NQUAD):
                    tqn = int(tq[b, q])
                    if tqn == 0:
                        continue
                    o16 = (t0 + off) * 8
                    nc.gpsimd.dma_gather(
                        out_ap=g[
                            :, off * ROWPAD : (off + tqn) * ROWPAD
                        ].rearrange("p (c d) -> p c d", d=ROWPAD),
                        in_ap=xq_p[q][:, :],
                        idxs_ap=idx_sb[:, o16 : o16 + tqn * 8],
                        num_idxs=tqn * P,
                        num_idxs_reg=int(vcnt[b, q]),
                        elem_size=ROWPAD,
                        queue_num=gather_i % 4,
                        single_packet=False,
                    )
                    gather_i += 1
                    off += tqn

                sel = sel_tp.tile([P, t_max * P], bf16, tag="s")
                nc.vector.tensor_tensor(
                    out=sel[:, : tb * P].rearrange("p (t n) -> p t n", n=P),
                    in0=col_sb[:, t0 : t0 + tb].unsqueeze(2).to_broadcast(
                        [P, tb, P]
                    ),
                    in1=iota_sb[:].unsqueeze(1).to_broadcast([P, tb, P]),
                    op=mybir.AluOpType.is_equal,
                )

                ps = psum_tp.tile([P, D_FEAT], f32, space="PSUM", tag="ps")
                for t in range(tb):
                    nc.tensor.matmul(
                        out=ps[:],
                        lhsT=sel[:, t * P : (t + 1) * P],
                        rhs=g[:, t * ROWPAD : t * ROWPAD + D_FEAT],
                        start=(t == 0),
                        stop=(t == tb - 1),
                    )

                xt = xin_tp.tile([P, D_FEAT], f32, tag="x")
                nc.sync.dma_start(out=xt[:rows], in_=xsl_p[b * P : b * P + rows, :])
                ot = osb_tp.tile([P, D_FEAT], f32, tag="o")
                nc.vector.tensor_add(out=ot[:rows], in0=xt[:rows], in1=ps[:rows])
                nc.sync.dma_start(out=out_p[b * P : b * P + rows, :], in_=ot[:rows])

                t0 += tb
    nc.compile()
    return nc


def kernel(x, edge_index):
    global LAST_EXEC_TIME_NS
    _patch_tile_drain()

    x = np.ascontiguousarray(np.asarray(x, dtype=np.float32))
    idx_arr, col_arr, tq, vcnt, t_total = _preprocess(x, edge_index)

    xq = np.zeros((N_NODES, ROWPAD), dtype=ml_dtypes.bfloat16)
    xq[:, :D_FEAT] = (x * np.float32(WEIGHT)).astype(ml_dtypes.bfloat16)
    iota = np.broadcast_to(
        np.arange(P, dtype=np.float32).astype(ml_dtypes.bfloat16), (P, P)
    ).copy()

    nc = _build_graph(tq, vcnt, t_total)

    in_maps = []
    for c in range(N_CORES):
        m = {
            "xsl": np.ascontiguousarray(x[c * NPC : (c + 1) * NPC]),
            "srci": np.ascontiguousarray(idx_arr[c]),
            "dcol": np.ascontiguousarray(col_arr[c]),
            "iota": iota,
        }
        for q in range(NQUAD):
            m[f"xq{q}"] = np.ascontiguousarray(
                xq[q * NQROWS : (q + 1) * NQROWS]
            )
        in_maps.append(m)

    trace = bool(os.environ.get("BASS_KERNEL_TRACE"))
    if trace:
        _install_ntff_hook()
    res = run_bass_kernel_spmd(
        nc, in_maps, core_ids=list(range(N_CORES)), trace=trace
    )
    LAST_EXEC_TIME_NS = res.exec_time_ns

    out = np.concatenate([res.results[c]["out"] for c in range(N_CORES)], axis=0)
    return out.astype(np.float32)


# revision 8
# speedup vs baseline: 3.7430x; 1.1015x over previous
"""Trainium2 Bass kernel for GNN message passing (APPR-style aggregation).

Computes: out = x + 0.15 * segment_sum(x[src], dst, num_segments=N)
for x [100000, 64] f32 and edge_index [2, 1600000] int64.

Strategy (8 NeuronCores, no collectives needed):
  - Host shards EDGES by destination-owner core (core c owns nodes
    [c*12500, (c+1)*12500)); within a core, edges are bucketed by
    128-node destination block and by source quadrant (x split into 4
    row-quadrants so dma_gather's int16 indices can address it). This
    makes the aggregation node-sharded from the start, so each core
    independently produces its slice of the output.
  - On device, per 128-node block: dma_gather of 0.15*x[src] rows (bf16,
    rows padded to 256 B) into SBUF, one gather per source quadrant,
    round-robined over all 4 SWDGE queues; build one-hot selection
    matrices S[e, j] = (dstcol_e == j) with a DVE is_equal against an
    iota tile, and accumulate S^T @ gathered into an f32 PSUM tile over
    all edge tiles of the block. Epilogue adds the f32 x slice and DMAs
    the block out.
  - All 8 cores run the same static graph: per-(block, quadrant) tile
    counts are maxed across cores; each core pads its edge list with
    valid index-0 entries up to the shared valid count and with -1
    entries (skipped by the gather) up to the tile boundary. Pad slots
    carry dstcol=255 which matches no one-hot column, so they contribute
    zero; slots the gather skips hold stale-but-finite SBUF data (the
    gather pool is memset once at startup).
"""

import os
import sys
import types

import numpy as np

for _p in ("/opt/trn_rl_repo", "/root/.axon_site/_ro/trn_rl_repo"):
    if os.path.isdir(_p) and _p not in sys.path:
        sys.path.append(_p)

import ml_dtypes
import concourse.bass as bass
import concourse.mybir as mybir
import concourse.tile as tile
from concourse import bacc
from concourse.bass_utils import run_bass_kernel_spmd
from concourse.vector_clock import ScopedClock

WEIGHT = 0.15
N_NODES = 100000
D_FEAT = 64
N_CORES = 8
P = 128
NQUAD = 4
ROWPAD = 128  # gathered bf16 row padded to 128 elems = 256 B
NPC = N_NODES // N_CORES  # nodes per core
NBLK = (NPC + P - 1) // P  # 128-node dst blocks per core
NQROWS = N_NODES // NQUAD  # rows per source quadrant (must fit int16)

LAST_EXEC_TIME_NS = None

# Emit only up to the max-over-cores valid count per gather and pad the
# rest with -1 (skipped; slots keep stale-but-finite SBUF data). The
# CoreSim interpreter poisons unwritten regions of the gather's out view,
# so simulation-based tests set this to False.
TRAILING_SKIP = True

MAX_WAITS = 2  # this walrus build rejects instructions with more sync commands


def _patch_tile_drain():
    """This walrus build rejects >MAX_WAITS sync commands (waits+updates)
    on one instruction. Two patches: (a) the tail drain re-emits its waits
    as individual wait_ge instructions; (b) any scheduled instruction with
    too many waits gets the excess hoisted onto same-engine InstNoOps
    placed immediately before it."""
    if getattr(tile.TileContext, "_drain_patched", False):
        return

    def _drain_and_barrier(self, tick_clock, wait_clock):
        drain_inst = self.nc.sync.drain()
        wait_clock.add_sem_waits(
            drain_inst.ins, ScopedClock({None: tick_clock.global_clock})
        )
        si = drain_inst.ins.sync_info
        waits = list(si.on_wait) if si is not None else []
        if len(waits) > MAX_WAITS:
            drain_inst.ins.sync_info = mybir.SyncInfo(on_wait=[], on_update=[])
            handles = {h.name: h for h in wait_clock.sems.allocated().values()}
            for w in waits:
                self.nc.sync.wait_ge(handles[w.ant_name], w.wait_value)
            self.nc.sync.drain()
        self.nc.all_engine_barrier()
        popped = self.nc._tile_sem_poison_stack.pop()
        assert popped is self._sem_poison
        self.nc.clear_and_free_semaphores(list(self.sems.allocated().values()))
        self.nc.all_engine_barrier()

    orig_lower = tile.TileContext._lower_ordered_insts

    def _lower_ordered_insts(self, ordered):
        for bb_name, insts in ordered.items():
            new_list = []
            for inst in insts:
                si = getattr(inst, "sync_info", None)
                n_w = len(si.on_wait) if si is not None and si.on_wait else 0
                n_u = len(si.on_update) if si is not None and si.on_update else 0
                budget = max(0, MAX_WAITS - n_u)
                if (
                    n_w > budget
                    and type(inst).__name__.startswith("Inst")
                    and inst.engine is not None
                ):
                    waits = list(si.on_wait)
                    keep = waits[len(waits) - budget :] if budget else []
                    excess = waits[: len(waits) - budget]
                    for w in excess:
                        nop = mybir.InstNoOp(
                            name=self.nc.get_next_instruction_name(),
                            sync_info=mybir.SyncInfo(on_wait=[w], on_update=[]),
                            engine=inst.engine,
                            bass_nofuse=True,
                        )
                        new_list.append(nop)
                    inst.sync_info = mybir.SyncInfo(
                        on_wait=keep, on_update=list(si.on_update)
                    )
                new_list.append(inst)
            insts[:] = new_list
        return orig_lower(self, ordered)

    tile.TileContext._drain_and_barrier = _drain_and_barrier
    tile.TileContext._lower_ordered_insts = _lower_ordered_insts
    tile.TileContext._drain_patched = True


def _install_ntff_hook():
    """Register the NTFF profiling hook that this container's boot skips
    (antenv.axon_hooks missing). Only needed when tracing is requested."""
    if "antenv.axon_hooks" in sys.modules:
        return
    try:
        from trn_agent_boot.trn_boot import _ntff_profile_via_ctypes

        hook = _ntff_profile_via_ctypes("/opt/axon/libaxon_pjrt.so")
        if hook is None:
            return
        mod = types.ModuleType("antenv.axon_hooks")
        mod._hook = hook
        mod.get_axon_ntff_profile_hook = lambda: mod._hook
        mod.set_axon_ntff_profile_hook = lambda h: setattr(mod, "_hook", h)
        sys.modules["antenv.axon_hooks"] = mod
        import antenv

        antenv.axon_hooks = mod
    except Exception as e:  # profiling is optional
        print(f"ntff hook install failed: {e}", file=sys.stderr)


def _preprocess(x, edge_index):
    """Bucket edges per (core, dst-block, src-quadrant); build device
    input arrays in the slot order the device graph consumes."""
    src = np.asarray(edge_index[0]).astype(np.int64)
    dst = np.asarray(edge_index[1]).astype(np.int64)
    E = src.shape[0]

    core = dst // NPC
    local = dst - core * NPC
    blk = local >> 7
    col = local & 127
    quad = src // NQROWS
    loc = (src - quad * NQROWS).astype(np.int64)

    gkey = (core * NBLK + blk) * NQUAD + quad
    order = np.argsort(gkey, kind="stable")
    gkey_s = gkey[order]
    loc_s = loc[order]
    col_s = col[order]

    counts = np.bincount(gkey, minlength=N_CORES * NBLK * NQUAD).reshape(
        N_CORES, NBLK, NQUAD
    )
    maxc = counts.max(axis=0)  # [NBLK, NQUAD] max edges over cores
    tq = (maxc + P - 1) // P  # tiles per (block, quadrant)
    for b in range(NBLK):
        if tq[b].sum() == 0:
            tq[b, 0] = 1
    tiles = tq.sum(axis=1)
    t_total = int(tiles.sum())
    if TRAILING_SKIP:
        vcnt = np.minimum((maxc + 15) // 16 * 16, tq * P)
        vcnt = np.maximum(vcnt, (tq > 0) * 16)
    else:
        vcnt = tq * P  # all slots valid (pad entries gather row 0)

    toff = np.zeros((NBLK, NQUAD), dtype=np.int64)
    toff.ravel()[1:] = np.cumsum(tq.ravel())[:-1]

    group_starts = np.zeros(N_CORES * NBLK * NQUAD + 1, dtype=np.int64)
    np.cumsum(counts.ravel(), out=group_starts[1:])
    j = np.arange(E) - group_starts[gkey_s]  # slot within group
    bq_s = gkey_s % (NBLK * NQUAD)
    core_s = gkey_s // (NBLK * NQUAD)
    base_tile = toff.ravel()[bq_s]

    # col array: [core, 128, t_total]; slot i -> lane i%128, tile base+i//128
    col_arr = np.full((N_CORES, P, t_total), 255.0, dtype=ml_dtypes.bfloat16)
    col_arr[core_s, j & 127, base_tile + (j >> 7)] = col_s.astype(
        ml_dtypes.bfloat16
    )

    # idx array (int16, wrapped in 16 partitions, replicated x8):
    # slot i of group -> [i%16, o16 + i//16] with o16 = base_tile*8.
    # Slots [count, vcnt) keep valid pad index 0 (dstcol=255 zeroes their
    # contribution); slots [vcnt, T*128) are -1 and skipped by the gather.
    t16_total = t_total * 8
    idx16 = np.zeros((N_CORES, 16, t16_total), dtype=np.int16)
    if TRAILING_SKIP:
        for b in range(NBLK):
            for q in range(NQUAD):
                v, tn = int(vcnt[b, q]), int(tq[b, q])
                if tn == 0 or v >= tn * P:
                    continue
                o16 = int(toff[b, q]) * 8
                full16 = v // 16  # v is a multiple of 16
                idx16[:, :, o16 + full16 : o16 + tn * 8] = -1
    idx16[core_s, j & 15, base_tile * 8 + (j >> 4)] = loc_s
    idx_arr = np.tile(idx16, (1, 8, 1))

    return idx_arr, col_arr, tq.astype(int), vcnt.astype(int), t_total


def _build_graph(tq, vcnt, t_total):
    nc = bacc.Bacc(num_swdge_queues=4)
    f32 = mybir.dt.float32
    bf16 = mybir.dt.bfloat16
    xq_p = [
        nc.declare_dram_parameter(f"xq{q}", [NQROWS, ROWPAD], bf16, isOutput=False)
        for q in range(NQUAD)
    ]
    xsl_p = nc.declare_dram_parameter("xsl", [NPC, D_FEAT], f32, isOutput=False)
    srci_p = nc.declare_dram_parameter(
        "srci", [P, t_total * 8], mybir.dt.int16, isOutput=False
    )
    dcol_p = nc.declare_dram_parameter("dcol", [P, t_total], bf16, isOutput=False)
    iota_p = nc.declare_dram_parameter("iota", [P, P], bf16, isOutput=False)
    out_p = nc.declare_dram_parameter("out", [NPC, D_FEAT], f32, isOutput=True)

    nblk = tq.shape[0]
    tiles = tq.sum(axis=1)
    t_max = int(tiles.max())

    with tile.TileContext(nc) as tc:
        with (
            tc.tile_pool(name="const", bufs=1) as const_tp,
            tc.tile_pool(name="meta", bufs=1) as meta_tp,
            tc.tile_pool(name="gather", bufs=4) as gather_tp,
            tc.tile_pool(name="sel", bufs=3) as sel_tp,
            tc.tile_pool(name="xin", bufs=3) as xin_tp,
            tc.tile_pool(name="osb", bufs=3) as osb_tp,
            tc.tile_pool(name="psum", bufs=4, space="PSUM") as psum_tp,
        ):
            iota_sb = const_tp.tile([P, P], bf16)
            nc.sync.dma_start(out=iota_sb[:], in_=iota_p[:])
            idx_sb = meta_tp.tile([P, t_total * 8], mybir.dt.int16)
            nc.sync.dma_start(out=idx_sb[:], in_=srci_p[:])
            col_sb = meta_tp.tile([P, t_total], bf16)
            nc.sync.dma_start(out=col_sb[:], in_=dcol_p[:])

            # memset the gather pool slots once: slots the -1-padded
            # gathers skip must hold finite data for the 0*x matmuls
            for _ in range(4):
                gz = gather_tp.tile([P, t_max * ROWPAD], bf16, tag="g")
                nc.vector.memset(gz[:], 0.0)

            gather_i = 0
            t0 = 0
            for b in range(nblk):
                tb = int(tiles[b])
                rows = min(P, NPC - b * P)

                g = gather_tp.tile([P, t_max * ROWPAD], bf16, tag="g")
                off = 0
                for q in range(NQUAD):
                    tqn = int(tq[b, q])
                    if tqn == 0:
                        continue
                    o16 = (t0 + off) * 8
                    nc.gpsimd.dma_gather(
                        out_ap=g[
                            :, off * ROWPAD : (off + tqn) * ROWPAD
                        ].rearrange("p (c d) -> p c d", d=ROWPAD),
                        in_ap=xq_p[q][:, :],
                        idxs_ap=idx_sb[:, o16 : o16 + tqn * 8],
                        num_idxs=tqn * P,
                        num_idxs_reg=int(vcnt[b, q]),
                        elem_size=ROWPAD,
                        queue_num=gather_i % 4,
                        single_packet=False,
                    )
                    gather_i += 1
                    off += tqn

                sel = sel_tp.tile([P, t_max * P], bf16, tag="s")
                nc.vector.tensor_tensor(
                    out=sel[:, : tb * P].rearrange("p (t n) -> p t n", n=P),
                    in0=col_sb[:, t0 : t0 + tb].unsqueeze(2).to_broadcast(
                        [P, tb, P]
                    ),
                    in1=iota_sb[:].unsqueeze(1).to_broadcast([P, tb, P]),
                    op=mybir.AluOpType.is_equal,
                )

                ps = psum_tp.tile([P, D_FEAT], f32, space="PSUM", tag="ps")
                for t in range(tb):
                    nc.tensor.matmul(
                        out=ps[:],
                        lhsT=sel[:, t * P : (t + 1) * P],
                        rhs=g[:, t * ROWPAD : t * ROWPAD + D_FEAT],
                        start=(t == 0),
                        stop=(t == tb - 1),
                    )

                xt = xin_tp.tile([P, D_FEAT], f32, tag="x")
                nc.sync.dma_start(out=xt[:rows], in_=xsl_p[b * P : b * P + rows, :])
                ot = osb_tp.tile([P, D_FEAT], f32, tag="o")
                nc.vector.tensor_add(out=ot[:rows], in0=xt[:rows], in1=ps[:rows])
                nc.sync.dma_start(out=out_p[b * P : b * P + rows, :], in_=ot[:rows])

                t0 += tb
    nc.compile()
    return nc


def kernel(x, edge_index):
    global LAST_EXEC_TIME_NS
    _patch_tile_drain()

    x = np.ascontiguousarray(np.asarray(x, dtype=np.float32))
    idx_arr, col_arr, tq, vcnt, t_total = _preprocess(x, edge_index)

    xq = np.zeros((N_NODES, ROWPAD), dtype=ml_dtypes.bfloat16)
    xq[:, :D_FEAT] = (x * np.float32(WEIGHT)).astype(ml_dtypes.bfloat16)
    iota = np.broadcast_to(
        np.arange(P, dtype=np.float32).astype(ml_dtypes.bfloat16), (P, P)
    ).copy()

    nc = _build_graph(tq, vcnt, t_total)

    in_maps = []
    for c in range(N_CORES):
        m = {
            "xsl": np.ascontiguousarray(x[c * NPC : (c + 1) * NPC]),
            "srci": np.ascontiguousarray(idx_arr[c]),
            "dcol": np.ascontiguousarray(col_arr[c]),
            "iota": iota,
        }
        for q in range(NQUAD):
            m[f"xq{q}"] = np.ascontiguousarray(
                xq[q * NQROWS : (q + 1) * NQROWS]
            )
        in_maps.append(m)

    trace = bool(os.environ.get("BASS_KERNEL_TRACE"))
    if trace:
        _install_ntff_hook()
    res = run_bass_kernel_spmd(
        nc, in_maps, core_ids=list(range(N_CORES)), trace=trace
    )
    LAST_EXEC_TIME_NS = res.exec_time_ns

    out = np.concatenate([res.results[c]["out"] for c in range(N_CORES)], axis=0)
    return out.astype(np.float32)
